# revision 19
# baseline (speedup 1.0000x reference)
"""Trainium2 Bass kernel for nn_ActorAgent (Mamba2-SSD actor network).

Data-parallel over batch: 32 sequences -> 8 cores x 4 sequences.
Per-core: full sequential SSM scan via tensor_tensor_scan on
[(h,p,n)=128, T] lanes, all per-batch selection/broadcast done with
constant block lhsT matmuls on the PE.
"""

import sys
import numpy as np

for _p in ("/opt/trn_rl_repo",):
    if _p not in sys.path:
        sys.path.append(_p)

import concourse.bass as bass
import concourse.bacc as bacc_mod
import concourse.mybir as mybir
from concourse.tile import TileContext
from concourse.bass_utils import run_bass_kernel_spmd

F32 = mybir.dt.float32
AF = mybir.ActivationFunctionType
ALU = mybir.AluOpType

T = 4096
B = 32
NCORES = 8
BL = B // NCORES          # 4 sequences per core
OBS = 32
DM = 16                   # d_model
DI = 16                   # d_inner
H = 8                     # heads
P = 2                     # headdim
N = 8                     # d_state
NA = 16                   # n_actions
NJ = 8                    # column tiles of 512
JW = 512


def _build_consts(W_in, b_in, in_proj, conv_w, conv_b, dt_bias, A_log, D_skip,
                  norm_w, out_proj, W_out, b_out):
    """Build the constant-weight blob [128, NCOLS] (fp32)."""
    A = -np.exp(A_log)

    cols = []
    offs = {}

    def add(name, m):
        # m: [K, M] -> pad partitions to 128
        k, w = m.shape
        pad = np.zeros((128, w), np.float32)
        pad[:k] = m
        offs[name] = (sum(c.shape[1] for c in cols), k, w)
        cols.append(pad)

    # xbc class-major M layout: m<64: xs (b=m//16, c=m%16); 64..96: B (b,n);
    # 96..128: C (b,n)
    def xbc_cm(m):
        if m < 64:
            return m // 16, m % 16
        if m < 96:
            return (m - 64) // 8, 16 + (m - 64) % 8
        return (m - 96) // 8, 24 + (m - 96) % 8

    # W1bd [128=(b,32f), 64=(b,16o)]
    w = np.zeros((128, 64), np.float32)
    for b in range(BL):
        w[32 * b:32 * b + 32, 16 * b:16 * b + 16] = W_in.T
    add("W1", w)

    # Wz [64=(b,16f), 64=(b,16z)]
    w = np.zeros((64, 64), np.float32)
    for b in range(BL):
        w[16 * b:16 * b + 16, 16 * b:16 * b + 16] = in_proj[:16].T
    add("Wz", w)

    # Wxbc [64=(b,16f), 128 class-major]
    w = np.zeros((64, 128), np.float32)
    for m in range(128):
        b, c = xbc_cm(m)
        w[16 * b:16 * b + 16, m] = in_proj[16 + c]
    add("Wxbc", w)

    # Wdt [64, 32=(b,8h)]
    w = np.zeros((64, 32), np.float32)
    for b in range(BL):
        w[16 * b:16 * b + 16, 8 * b:8 * b + 8] = in_proj[48:56].T
    add("Wdt", w)

    # E_dtr [32=(b,h), 64=(b,hp)] (x0.5: xs is stored as 2*silu)
    w = np.zeros((32, 64), np.float32)
    for m in range(64):
        b, hp = m // 16, m % 16
        w[8 * b + hp // 2, m] = 0.5
    add("Edtr", w)

    # per-b selectors, m=(h,p,n): h=m//16, hp=m//8, n=m%8
    for b in range(BL):
        w = np.zeros((64, 128), np.float32)   # E_dtx_b: k=(b',hp16)
        for m in range(128):
            w[16 * b + m // 8, m] = 1.0
        add(f"Edtx{b}", w)
    for b in range(BL):
        w = np.zeros((128, 128), np.float32)  # E_B_b from xbc_c rows (x0.5)
        for m in range(128):
            w[64 + 8 * b + m % 8, m] = 0.5
        add(f"EB{b}", w)
    for b in range(BL):
        w = np.zeros((128, 128), np.float32)  # E_C_b (x0.5)
        for m in range(128):
            w[96 + 8 * b + m % 8, m] = 0.5
        add(f"EC{b}", w)
    for b in range(BL):
        w = np.zeros((32, 128), np.float32)   # E_da_b from da_all [32=(b,h)]
        for m in range(128):
            w[8 * b + m // 16, m] = 1.0
        add(f"Eda{b}", w)
    for b in range(BL):
        w = np.zeros((128, 64), np.float32)   # Wred_b: sum over n -> (b,hp)
        for k in range(128):
            w[k, 16 * b + k // 8] = 1.0
        add(f"Wred{b}", w)

    # Wsq [64=(b,hp), 4]
    w = np.zeros((64, 4), np.float32)
    for k in range(64):
        w[k, k // 16] = 1.0
    add("Wsq", w)
    # E4b [4, 64]
    w = np.zeros((4, 64), np.float32)
    for m in range(64):
        w[m // 16, m] = 1.0
    add("E4b", w)
    # per-j selectors from rsq_pack [32=(j,b), .] -> [64=(b,hp), .]
    for j in range(8):
        w = np.zeros((32, 64), np.float32)
        for m in range(64):
            w[4 * j + m // 16, m] = 1.0
        add(f"E4b{j}", w)
    # OutP [64=(b,hp), 64=(b,d)] with norm_w folded
    opw = out_proj * norm_w[None, :]          # [16d, 16hp]
    w = np.zeros((64, 64), np.float32)
    for b in range(BL):
        w[16 * b:16 * b + 16, 16 * b:16 * b + 16] = opw.T
    add("OutP", w)
    # WoutT [65, 64=(b,a)]: rhs for t-major logits
    w = np.zeros((65, 64), np.float32)
    for b in range(BL):
        w[16 * b:16 * b + 16, 16 * b:16 * b + 16] = W_out.T
    w[64] = np.tile(b_out, BL)
    add("WoutT", w)

    # scalar columns
    cvec = np.array([xbc_cm(m)[1] for m in range(128)])
    add("w0c", conv_w[cvec, 0][:, None].astype(np.float32))
    add("w1c", conv_w[cvec, 1][:, None].astype(np.float32))
    add("cbc", conv_b[cvec][:, None].astype(np.float32))
    add("binc", np.tile(b_in, BL)[:, None].astype(np.float32))
    add("dtbc", np.tile(dt_bias, BL)[:, None].astype(np.float32))
    add("Ac", np.tile(A, BL)[:, None].astype(np.float32))
    add("Dskc", 0.5 * np.tile(np.repeat(D_skip, P), BL)[:, None].astype(np.float32))
    add("zeroc", np.zeros((128, 1), np.float32))
    add("epsc", np.full((128, 1), 4e-5, np.float32))
    add("onec", np.ones((128, 1), np.float32))

    blob = np.concatenate(cols, axis=1).astype(np.float32)
    return blob, offs


def build_nc(offs, ncols):
    nc = bacc_mod.Bacc()
    obs_t = nc.declare_dram_parameter("obs_t", [128, T], F32, isOutput=False)
    cw = nc.declare_dram_parameter("cw", [128, ncols], F32, isOutput=False)
    out = nc.declare_dram_parameter("out", [T, BL, NA], F32, isOutput=True)
    BF16 = mybir.dt.bfloat16

    with TileContext(nc) as tc:
        with (
            tc.tile_pool(name="persist", bufs=1) as pp,
            tc.tile_pool(name="sb", bufs=2) as sb,
            tc.tile_pool(name="sbA", bufs=1) as sbA,
            tc.tile_pool(name="sb2", bufs=2) as sb2,
            tc.tile_pool(name="hpool", bufs=3) as hpool,
            tc.tile_pool(name="obsp", bufs=2) as obsp,
            tc.tile_pool(name="xbcp", bufs=2) as xbcp,
            tc.tile_pool(name="psA", bufs=4, space="PSUM") as psA,
            tc.tile_pool(name="psY", bufs=1, space="PSUM") as psY,
            tc.tile_pool(name="psS", bufs=1, space="PSUM") as psS,
        ):
            cwt = pp.tile([128, ncols], F32, tag="cw")
            nc.sync.dma_start(cwt[:, :], cw[:, :])

            def CW(name, nk=None):
                c0, k, w = offs[name]
                return cwt[0:(nk or k), c0:c0 + w]

            x_all = pp.tile([64, T], F32, tag="x_all")
            dt_all = pp.tile([32, T], F32, tag="dt_all")
            da_all = pp.tile([32, T], F32, tag="da_all")
            y3_all = pp.tile([64, T], BF16, tag="y3_all")
            ms_cp = pp.tile([4, T], F32, tag="ms_cp")
            hlast = []
            for b in range(BL):
                hl_b = pp.tile([128, 1], F32, tag=f"hlast{b}")
                hlast.append(hl_b)

            # dummy matmul: folds the cwt DMA wait into PE's clock so the
            # first real matmul carries a single sync wait (walrus limit)
            ps_w = psS.tile([1, 16], F32, tag="pss")
            nc.tensor.matmul(ps_w[0:1, :], cwt[0:1, 0:1], cwt[0:1, 0:16])

            # ================= phase 0: x, dt, da  (ACT set: ln+exp) ======
            for j in range(NJ):
                c0 = j * JW
                ot = obsp.tile([128, JW], F32, tag="obs")
                nc.sync.dma_start(ot[:, :], obs_t[:, c0:c0 + JW])
                ps = psA.tile([128, JW], F32, tag="psa")
                nc.tensor.matmul(ps[0:64, :], CW("W1"), ot[:, :])
                nc.scalar.activation(x_all[:, c0:c0 + JW], ps[0:64, :],
                                     AF.Relu, bias=CW("binc"))
                ps = psA.tile([128, JW], F32, tag="psa")
                nc.tensor.matmul(ps[0:32, :], CW("Wdt"),
                                 x_all[:, c0:c0 + JW])
                # softplus(v) = relu(v) + ln(1 + exp(-|v|)), v = raw + dt_bias
                rl = sbA.tile([32, JW], F32, tag="spt1")
                nc.scalar.activation(rl[:, :], ps[0:32, :],
                                     AF.Relu, bias=CW("dtbc", 32))
                ab = sbA.tile([32, JW], F32, tag="spt2")
                nc.scalar.activation(ab[:, :], ps[0:32, :],
                                     AF.Abs, bias=CW("dtbc", 32))
                et = sbA.tile([32, JW], F32, tag="spt3")
                nc.scalar.activation(et[:, :], ab[:, :], AF.Exp,
                                     bias=CW("zeroc", 32), scale=-1.0)
                lt = sbA.tile([32, JW], F32, tag="spt4")
                nc.scalar.activation(lt[:, :], et[:, :], AF.Ln,
                                     bias=CW("onec", 32))
                nc.vector.tensor_add(dt_all[:, c0:c0 + JW], rl[:, :],
                                     lt[:, :])
                adt = sbA.tile([32, JW], F32, tag="spt5")
                nc.vector.tensor_scalar_mul(adt[:, :],
                                            dt_all[:, c0:c0 + JW],
                                            CW("Ac", 32))
                nc.scalar.activation(da_all[:, c0:c0 + JW], adt[:, :],
                                     AF.Exp, bias=CW("zeroc", 32))

            # ====== phase 1: z, conv, scan, gate  (ACT set: exp+tanh) =====
            prev_xbc = None
            for j in range(NJ):
                c0 = j * JW

                ps_z = psA.tile([128, JW], F32, tag="psa")
                nc.tensor.matmul(ps_z[0:64, :], CW("Wz"), x_all[:, c0:c0 + JW])
                th_z = sbA.tile([64, JW], F32, tag="thz")
                nc.scalar.activation(th_z[:, :], ps_z[0:64, :], AF.Tanh,
                                     bias=CW("zeroc", 64), scale=0.5)
                z_sb = sbA.tile([64, JW], F32, tag="zsb")
                nc.scalar.copy(z_sb[:, :], ps_z[0:64, :])
                zs_j = sb.tile([64, JW], F32, tag="zs")
                nc.vector.scalar_tensor_tensor(zs_j[:, :], th_z[:, :], 1.0,
                                               z_sb[:, :], ALU.add, ALU.mult)

                xbc_r = xbcp.tile([128, 8 + JW], F32, tag="xbcraw")
                if j == 0:
                    nc.vector.memset(xbc_r[:, 0:8], 0.0)
                else:
                    nc.vector.tensor_copy(xbc_r[:, 0:8],
                                          prev_xbc[:, JW:JW + 8])
                ps = psA.tile([128, JW], F32, tag="psa")
                nc.tensor.matmul(ps[:, :], CW("Wxbc"), x_all[:, c0:c0 + JW])
                nc.scalar.copy(xbc_r[:, 8:8 + JW], ps[:, :])
                prev_xbc = xbc_r

                t1 = sb.tile([128, JW], F32, tag="conv1")
                nc.gpsimd.tensor_scalar(t1[:, :], xbc_r[:, 8:8 + JW],
                                        CW("w1c"), CW("cbc"),
                                        ALU.mult, ALU.add)  # gpsimd ts ok?
                vj = sb.tile([128, JW], F32, tag="conv2")
                nc.vector.scalar_tensor_tensor(vj[:, :], xbc_r[:, 7:7 + JW],
                                               CW("w0c"), t1[:, :],
                                               ALU.mult, ALU.add)
                th_c = sbA.tile([128, JW], F32, tag="thc")
                nc.scalar.activation(th_c[:, :], vj[:, :], AF.Tanh,
                                     bias=CW("zeroc"), scale=0.5)
                xbc_j = sb.tile([128, JW], F32, tag="xbcc")
                nc.vector.scalar_tensor_tensor(xbc_j[:, :], th_c[:, :], 1.0,
                                               vj[:, :], ALU.add, ALU.mult)

                ps_dtr = psA.tile([128, JW], F32, tag="psa")
                nc.tensor.matmul(ps_dtr[0:64, :], CW("Edtr"),
                                 dt_all[:, c0:c0 + JW])
                dtx_j = sb2.tile([64, JW], F32, tag="dtx")
                nc.vector.tensor_mul(dtx_j[:, :], xbc_j[0:64, :],
                                     ps_dtr[0:64, :])

                ps_y = psY.tile([64, JW], F32, tag="psy")
                for b in range(BL):
                    ps_da = psA.tile([128, JW], F32, tag="psa")
                    nc.tensor.matmul(ps_da[:, :], CW(f"Eda{b}"),
                                     da_all[:, c0:c0 + JW])
                    ps_dtx = psA.tile([128, JW], F32, tag="psa")
                    nc.tensor.matmul(ps_dtx[:, :], CW(f"Edtx{b}"),
                                     dtx_j[:, :])
                    ps_B = psA.tile([128, JW], F32, tag="psa")
                    nc.tensor.matmul(ps_B[:, :], CW(f"EB{b}"), xbc_j[:, :])
                    B_sb = sb2.tile([128, JW], F32, tag="Bsb")
                    nc.scalar.copy(B_sb[:, :], ps_B[:, :])
                    d1 = sb2.tile([128, JW], F32, tag="d1")
                    nc.vector.tensor_mul(d1[:, :], B_sb[:, :], ps_dtx[:, :])

                    h_bj = hpool.tile([128, JW], F32, tag="h")
                    init = 0.0 if j == 0 else hlast[b][:, 0:1]
                    nc.vector.tensor_tensor_scan(h_bj[:, :], ps_da[:, :],
                                                 d1[:, :], init,
                                                 ALU.mult, ALU.add)
                    nc.vector.tensor_copy(hlast[b][:, 0:1],
                                          h_bj[:, JW - 1:JW])

                    ps_C = psA.tile([128, JW], F32, tag="psa")
                    nc.tensor.matmul(ps_C[:, :], CW(f"EC{b}"), xbc_j[:, :])
                    Pm = sb2.tile([128, JW], F32, tag="Pm")
                    nc.vector.tensor_mul(Pm[:, :], h_bj[:, :], ps_C[:, :])
                    nc.tensor.matmul(ps_y[:, :], CW(f"Wred{b}"), Pm[:, :],
                                     start=(b == 0), stop=(b == BL - 1))

                y2 = sb.tile([64, JW], F32, tag="y2")
                nc.vector.scalar_tensor_tensor(y2[:, :], xbc_j[0:64, :],
                                               CW("Dskc"), ps_y[:, :],
                                               ALU.mult, ALU.add)
                nc.vector.tensor_mul(y3_all[:, c0:c0 + JW], y2[:, :],
                                     zs_j[:, :])
                sq = sbA.tile([64, JW], F32, tag="sq")
                nc.scalar.activation(sq[:, :], y3_all[:, c0:c0 + JW],
                                     AF.Square, bias=CW("zeroc", 64))
                ps_s = psS.tile([4, JW], F32, tag="pss")
                nc.tensor.matmul(ps_s[:, :], CW("Wsq"), sq[:, :])
                nc.scalar.copy(ms_cp[:, c0:c0 + JW], ps_s[:, :])

            # ====== phase 2: rsqrt via exp(-0.5 ln(.))  (ACT: ln+exp) =====
            for g in range(2):
                g0 = g * (T // 2)
                gw = T // 2
                ln_t = sbA.tile([4, gw], F32, tag="ln_t")
                nc.scalar.activation(ln_t[:, :], ms_cp[:, g0:g0 + gw], AF.Ln,
                                     bias=CW("epsc", 4), scale=1.0 / DI)
                nc.scalar.activation(ms_cp[:, g0:g0 + gw], ln_t[:, :],
                                     AF.Exp, bias=CW("zeroc", 4), scale=-0.5)

            # ====== phase 3: y4, out_proj, logits (ACT: relu/copy any) ====
            for j in range(NJ):
                c0 = j * JW
                ps_r = psS.tile([64, JW], F32, tag="psr")
                nc.tensor.matmul(ps_r[:, :], CW("E4b"),
                                 ms_cp[:, c0:c0 + JW])
                y4 = sb.tile([64, JW], F32, tag="y4")
                nc.vector.tensor_mul(y4[:, :], y3_all[:, c0:c0 + JW],
                                     ps_r[:, :])
                ps_yo = psY.tile([64, JW], F32, tag="psyo")
                nc.tensor.matmul(ps_yo[:, :], CW("OutP"), y4[:, :])
                hid = sbA.tile([65, JW], F32, tag="hid")
                nc.scalar.activation(hid[0:64, :], ps_yo[:, :], AF.Relu,
                                     bias=CW("zeroc", 64))
                nc.gpsimd.memset(hid[64:65, :], 1.0)

                stage = sbA.tile([128, 256], F32, tag="stage")
                for blk in range(4):
                    ps_lg = psS.tile([128, 64], F32, tag="psr")
                    nc.tensor.matmul(ps_lg[:, :],
                                     hid[:, 128 * blk:128 * blk + 128],
                                     CW("WoutT"))
                    if blk % 2 == 0:
                        nc.vector.tensor_copy(stage[:, 64 * blk:64 * blk + 64],
                                              ps_lg[:, :])
                    else:
                        nc.scalar.copy(stage[:, 64 * blk:64 * blk + 64],
                                       ps_lg[:, :])

                out_v = out.ap()[c0:c0 + JW, :, :].rearrange(
                    "(k t) b a -> t k (b a)", t=128)
                nc.sync.dma_start(out_v, stage[:, :].rearrange(
                    "p (k a) -> p k a", k=4))

    nc.compile()
    return nc


_CACHE = {}


def kernel(**inputs):
    obs = np.asarray(inputs["obs"], np.float32)
    blob, offs = _build_consts(
        np.asarray(inputs["W_in"], np.float32),
        np.asarray(inputs["b_in"], np.float32),
        np.asarray(inputs["in_proj"], np.float32),
        np.asarray(inputs["conv_w"], np.float32),
        np.asarray(inputs["conv_b"], np.float32),
        np.asarray(inputs["dt_bias"], np.float32),
        np.asarray(inputs["A_log"], np.float32),
        np.asarray(inputs["D_skip"], np.float32),
        np.asarray(inputs["norm_w"], np.float32),
        np.asarray(inputs["out_proj"], np.float32),
        np.asarray(inputs["W_out"], np.float32),
        np.asarray(inputs["b_out"], np.float32),
    )

    if "nc" not in _CACHE:
        _CACHE["nc"] = build_nc(offs, blob.shape[1])
    nc = _CACHE["nc"]

    # shard obs: core i gets batches [4i, 4i+4): [T, 32b, 32f] -> [(b,f)=128, T]
    obs_bft = np.ascontiguousarray(obs.transpose(1, 2, 0))  # [32b, 32f, T]
    in_maps = []
    for i in range(NCORES):
        shard = obs_bft[BL * i:BL * i + BL].reshape(128, T)
        in_maps.append({"obs_t": np.ascontiguousarray(shard), "cw": blob})

    res = run_bass_kernel_spmd(nc, in_maps, core_ids=list(range(NCORES)))
    outs = [res.results[i]["out"] for i in range(NCORES)]
    return np.concatenate(outs, axis=1)  # [T, 32, 16]


def _install_ntff_hook():
    """Provide antenv.axon_hooks with a ctypes NTFF profile hook (missing
    in this container image) so run_bass_kernel_spmd(trace=True) works."""
    import sys as _sys
    import types as _types
    import contextlib as _ctx
    import ctypes as _ct
    if "antenv.axon_hooks" in _sys.modules:
        return
    try:
        lib = _ct.CDLL("/opt/axon/libaxon_pjrt.so")
        if not hasattr(lib, "axon_start_nrt_profile"):
            return
    except OSError:
        return
    lib.axon_start_nrt_profile.argtypes = [_ct.POINTER(_ct.c_int64),
                                           _ct.c_size_t]
    lib.axon_start_nrt_profile.restype = _ct.c_int64
    lib.axon_stop_nrt_profile.argtypes = [_ct.c_char_p]
    lib.axon_stop_nrt_profile.restype = _ct.c_int64

    @_ctx.contextmanager
    def _hook(output_dir, device_ids):
        import jax
        jax.devices()
        if device_ids:
            ids = (_ct.c_int64 * len(device_ids))(*device_ids)
            rc = lib.axon_start_nrt_profile(ids, len(device_ids))
        else:
            rc = lib.axon_start_nrt_profile(None, 0)
        if rc != 0:
            raise RuntimeError(f"axon_start_nrt_profile rc={rc}")
        try:
            yield
        finally:
            n = lib.axon_stop_nrt_profile(str(output_dir).encode())
            print(f"ntff profile: {n} file(s) -> {output_dir}")

    mod = _types.ModuleType("antenv.axon_hooks")
    mod.get_axon_ntff_profile_hook = lambda: _hook
    mod.set_axon_ntff_profile_hook = lambda h: None
    _sys.modules["antenv.axon_hooks"] = mod
    import antenv
    antenv.axon_hooks = mod


def bench(**inputs):
    """Correctness + profiled run; returns (out, exec_time_ns)."""
    out = kernel(**inputs)  # ensures _CACHE["nc"] built with same consts
    obs = np.asarray(inputs["obs"], np.float32)
    blob, offs = _build_consts(*[np.asarray(inputs[k], np.float32) for k in (
        "W_in", "b_in", "in_proj", "conv_w", "conv_b", "dt_bias", "A_log",
        "D_skip", "norm_w", "out_proj", "W_out", "b_out")])
    obs_bft = np.ascontiguousarray(obs.transpose(1, 2, 0))
    in_maps = []
    for i in range(NCORES):
        shard = obs_bft[BL * i:BL * i + BL].reshape(128, T)
        in_maps.append({"obs_t": np.ascontiguousarray(shard), "cw": blob})
    _install_ntff_hook()
    try:
        res = run_bass_kernel_spmd(_CACHE["nc"], in_maps,
                                   core_ids=list(range(NCORES)), trace=True)
        return out, res.exec_time_ns, res
    except Exception as e:
        print("trace run failed:", repr(e)[:300])
        return out, None, None


# revision 20
# speedup vs baseline: 1.3070x; 1.3070x over previous
"""Trainium2 Bass kernel for nn_ActorAgent (Mamba2-SSD actor network).

Data-parallel over batch: 32 sequences -> 8 cores x 4 sequences.
Per-core: full sequential SSM scan via tensor_tensor_scan on
[(h,p,n)=128, T] lanes, all per-batch selection/broadcast done with
constant block lhsT matmuls on the PE.
"""

import sys
import numpy as np
import ml_dtypes

for _p in ("/opt/trn_rl_repo",):
    if _p not in sys.path:
        sys.path.append(_p)

import concourse.bass as bass
import concourse.bacc as bacc_mod
import concourse.mybir as mybir
from concourse.tile import TileContext
from concourse.bass_utils import run_bass_kernel_spmd

F32 = mybir.dt.float32
AF = mybir.ActivationFunctionType
ALU = mybir.AluOpType

T = 4096
B = 32
NCORES = 8
BL = B // NCORES          # 4 sequences per core
OBS = 32
DM = 16                   # d_model
DI = 16                   # d_inner
H = 8                     # heads
P = 2                     # headdim
N = 8                     # d_state
NA = 16                   # n_actions
NJ = 8                    # column tiles of 512
JW = 512


def _build_consts(W_in, b_in, in_proj, conv_w, conv_b, dt_bias, A_log, D_skip,
                  norm_w, out_proj, W_out, b_out):
    """Build the constant-weight blob [128, NCOLS] (fp32)."""
    A = -np.exp(A_log)

    cols = []
    offs = {}

    def add(name, m):
        # m: [K, M] -> pad partitions to 128
        k, w = m.shape
        pad = np.zeros((128, w), np.float32)
        pad[:k] = m
        offs[name] = (sum(c.shape[1] for c in cols), k, w)
        cols.append(pad)

    # xbc class-major M layout: m<64: xs (b=m//16, c=m%16); 64..96: B (b,n);
    # 96..128: C (b,n)
    def xbc_cm(m):
        if m < 64:
            return m // 16, m % 16
        if m < 96:
            return (m - 64) // 8, 16 + (m - 64) % 8
        return (m - 96) // 8, 24 + (m - 96) % 8

    # W1bd [128=(b,32f), 64=(b,16o)]
    w = np.zeros((128, 64), np.float32)
    for b in range(BL):
        w[32 * b:32 * b + 32, 16 * b:16 * b + 16] = W_in.T
    add("W1", w)

    # Wz [64=(b,16f), 64=(b,16z)]
    w = np.zeros((64, 64), np.float32)
    for b in range(BL):
        w[16 * b:16 * b + 16, 16 * b:16 * b + 16] = in_proj[:16].T
    add("Wz", w)

    # Wxbc [64=(b,16f), 128 class-major]
    w = np.zeros((64, 128), np.float32)
    for m in range(128):
        b, c = xbc_cm(m)
        w[16 * b:16 * b + 16, m] = in_proj[16 + c]
    add("Wxbc", w)

    # Wdt [64, 32=(b,8h)]
    w = np.zeros((64, 32), np.float32)
    for b in range(BL):
        w[16 * b:16 * b + 16, 8 * b:8 * b + 8] = in_proj[48:56].T
    add("Wdt", w)

    # E_dtr [32=(b,h), 64=(b,hp)] (x0.5: xs is stored as 2*silu)
    w = np.zeros((32, 64), np.float32)
    for m in range(64):
        b, hp = m // 16, m % 16
        w[8 * b + hp // 2, m] = 0.5
    add("Edtr", w)

    # per-b selectors, m=(h,p,n): h=m//16, hp=m//8, n=m%8
    for b in range(BL):
        w = np.zeros((64, 128), np.float32)   # E_dtx_b: k=(b',hp16)
        for m in range(128):
            w[16 * b + m // 8, m] = 1.0
        add(f"Edtx{b}", w)
    for b in range(BL):
        w = np.zeros((128, 128), np.float32)  # E_B_b from xbc_c rows (x0.5)
        for m in range(128):
            w[64 + 8 * b + m % 8, m] = 0.5
        add(f"EB{b}", w)
    for b in range(BL):
        w = np.zeros((128, 128), np.float32)  # E_C_b (x0.5)
        for m in range(128):
            w[96 + 8 * b + m % 8, m] = 0.5
        add(f"EC{b}", w)
    for b in range(BL):
        w = np.zeros((32, 128), np.float32)   # E_da_b from da_all [32=(b,h)]
        for m in range(128):
            w[8 * b + m // 16, m] = 1.0
        add(f"Eda{b}", w)
    for b in range(BL):
        w = np.zeros((128, 64), np.float32)   # Wred_b: sum over n -> (b,hp)
        for k in range(128):
            w[k, 16 * b + k // 8] = 1.0
        add(f"Wred{b}", w)

    # Wsq [64=(b,hp), 4]
    w = np.zeros((64, 4), np.float32)
    for k in range(64):
        w[k, k // 16] = 1.0
    add("Wsq", w)
    # E4b [4, 64]
    w = np.zeros((4, 64), np.float32)
    for m in range(64):
        w[m // 16, m] = 1.0
    add("E4b", w)
    # per-j selectors from rsq_pack [32=(j,b), .] -> [64=(b,hp), .]
    for j in range(8):
        w = np.zeros((32, 64), np.float32)
        for m in range(64):
            w[4 * j + m // 16, m] = 1.0
        add(f"E4b{j}", w)
    # OutP [64=(b,hp), 64=(b,d)] with norm_w folded
    opw = out_proj * norm_w[None, :]          # [16d, 16hp]
    w = np.zeros((64, 64), np.float32)
    for b in range(BL):
        w[16 * b:16 * b + 16, 16 * b:16 * b + 16] = opw.T
    add("OutP", w)
    # WoutT [65, 64=(b,a)]: rhs for t-major logits
    w = np.zeros((65, 64), np.float32)
    for b in range(BL):
        w[16 * b:16 * b + 16, 16 * b:16 * b + 16] = W_out.T
    w[64] = np.tile(b_out, BL)
    add("WoutT", w)

    # scalar columns
    cvec = np.array([xbc_cm(m)[1] for m in range(128)])
    add("w0c", conv_w[cvec, 0][:, None].astype(np.float32))
    add("w1c", conv_w[cvec, 1][:, None].astype(np.float32))
    add("cbc", conv_b[cvec][:, None].astype(np.float32))
    add("binc", np.tile(b_in, BL)[:, None].astype(np.float32))
    add("dtbc", np.tile(dt_bias, BL)[:, None].astype(np.float32))
    add("Ac", np.tile(A, BL)[:, None].astype(np.float32))
    add("Dskc", 0.5 * np.tile(np.repeat(D_skip, P), BL)[:, None].astype(np.float32))
    add("zeroc", np.zeros((128, 1), np.float32))
    add("epsc", np.full((128, 1), 4e-5, np.float32))
    add("onec", np.ones((128, 1), np.float32))

    blob = np.concatenate(cols, axis=1).astype(np.float32)
    return blob, offs


def build_nc(offs, ncols):
    nc = bacc_mod.Bacc()
    BF = mybir.dt.bfloat16
    obs_t = nc.declare_dram_parameter("obs_t", [128, T], BF, isOutput=False)
    cw = nc.declare_dram_parameter("cw", [128, ncols], F32, isOutput=False)
    cwb = nc.declare_dram_parameter("cwb", [128, ncols], BF, isOutput=False)
    out = nc.declare_dram_parameter("out", [T, BL, NA], F32, isOutput=True)
    BF16 = mybir.dt.bfloat16

    with TileContext(nc) as tc:
        with (
            tc.tile_pool(name="persist", bufs=1) as pp,
            tc.tile_pool(name="sb", bufs=2) as sb,
            tc.tile_pool(name="sbA", bufs=1) as sbA,
            tc.tile_pool(name="sb2", bufs=2) as sb2,
            tc.tile_pool(name="hpool", bufs=3) as hpool,
            tc.tile_pool(name="obsp", bufs=2) as obsp,
            tc.tile_pool(name="xbcp", bufs=2) as xbcp,
            tc.tile_pool(name="psA", bufs=4, space="PSUM") as psA,
            tc.tile_pool(name="psY", bufs=1, space="PSUM") as psY,
            tc.tile_pool(name="psS", bufs=1, space="PSUM") as psS,
        ):
            cwt = pp.tile([128, ncols], F32, tag="cw")
            nc.sync.dma_start(cwt[:, :], cw[:, :])
            cwbt = pp.tile([128, ncols], BF16, tag="cwb")
            nc.sync.dma_start(cwbt[:, :], cwb[:, :])

            def CW(name, nk=None):
                c0, k, w = offs[name]
                return cwt[0:(nk or k), c0:c0 + w]

            def CWB(name, nk=None):
                c0, k, w = offs[name]
                return cwbt[0:(nk or k), c0:c0 + w]

            x_all = pp.tile([64, T], BF16, tag="x_all")
            dt_all = pp.tile([32, T], BF16, tag="dt_all")
            da_all = pp.tile([32, T], F32, tag="da_all")
            y3_all = pp.tile([64, T], BF16, tag="y3_all")
            ms_cp = pp.tile([4, T], BF16, tag="ms_cp")
            hlast = []
            for b in range(BL):
                hl_b = pp.tile([128, 1], F32, tag=f"hlast{b}")
                hlast.append(hl_b)

            # dummy matmul: folds the cwt DMA wait into PE's clock so the
            # first real matmul carries a single sync wait (walrus limit)
            ps_w = psS.tile([1, 16], F32, tag="pss")
            nc.tensor.matmul(ps_w[0:1, :], cwt[0:1, 0:1], cwt[0:1, 0:16])

            # ================= phase 0: x, dt, da  (ACT set: ln+exp) ======
            for j in range(NJ):
                c0 = j * JW
                ot = obsp.tile([128, JW], BF16, tag="obs")
                nc.sync.dma_start(ot[:, :], obs_t[:, c0:c0 + JW])
                ps = psA.tile([128, JW], F32, tag="psa")
                nc.tensor.matmul(ps[0:64, :], CWB("W1"), ot[:, :])
                nc.scalar.activation(x_all[:, c0:c0 + JW], ps[0:64, :],
                                     AF.Relu, bias=CW("binc"))
                ps = psA.tile([128, JW], F32, tag="psa")
                nc.tensor.matmul(ps[0:32, :], CWB("Wdt"),
                                 x_all[:, c0:c0 + JW])
                # softplus(v) = relu(v) + ln(1 + exp(-|v|)), v = raw + dt_bias
                rl = sbA.tile([32, JW], F32, tag="spt1")
                nc.scalar.activation(rl[:, :], ps[0:32, :],
                                     AF.Relu, bias=CW("dtbc", 32))
                ab = sbA.tile([32, JW], F32, tag="spt2")
                nc.scalar.activation(ab[:, :], ps[0:32, :],
                                     AF.Abs, bias=CW("dtbc", 32))
                et = sbA.tile([32, JW], F32, tag="spt3")
                nc.scalar.activation(et[:, :], ab[:, :], AF.Exp,
                                     bias=CW("zeroc", 32), scale=-1.0)
                lt = sbA.tile([32, JW], F32, tag="spt4")
                nc.scalar.activation(lt[:, :], et[:, :], AF.Ln,
                                     bias=CW("onec", 32))
                nc.vector.tensor_add(dt_all[:, c0:c0 + JW], rl[:, :],
                                     lt[:, :])
                adt = sbA.tile([32, JW], F32, tag="spt5")
                nc.vector.tensor_scalar_mul(adt[:, :],
                                            dt_all[:, c0:c0 + JW],
                                            CW("Ac", 32))
                nc.scalar.activation(da_all[:, c0:c0 + JW], adt[:, :],
                                     AF.Exp, bias=CW("zeroc", 32))

            # ====== phase 1: z, conv, scan, gate  (ACT set: exp+tanh) =====
            prev_xbc = None
            for j in range(NJ):
                c0 = j * JW

                ps_z = psA.tile([128, JW], F32, tag="psa")
                nc.tensor.matmul(ps_z[0:64, :], CWB("Wz"), x_all[:, c0:c0 + JW])
                th_z = sbA.tile([64, JW], F32, tag="thz")
                nc.scalar.activation(th_z[:, :], ps_z[0:64, :], AF.Tanh,
                                     bias=CW("zeroc", 64), scale=0.5)
                z_sb = sbA.tile([64, JW], F32, tag="zsb")
                nc.scalar.copy(z_sb[:, :], ps_z[0:64, :])
                zs_j = sb.tile([64, JW], F32, tag="zs")
                nc.vector.scalar_tensor_tensor(zs_j[:, :], th_z[:, :], 1.0,
                                               z_sb[:, :], ALU.add, ALU.mult)

                xbc_r = xbcp.tile([128, 8 + JW], F32, tag="xbcraw")
                if j == 0:
                    nc.vector.memset(xbc_r[:, 0:8], 0.0)
                else:
                    nc.vector.tensor_copy(xbc_r[:, 0:8],
                                          prev_xbc[:, JW:JW + 8])
                ps = psA.tile([128, JW], F32, tag="psa")
                nc.tensor.matmul(ps[:, :], CWB("Wxbc"), x_all[:, c0:c0 + JW])
                nc.scalar.copy(xbc_r[:, 8:8 + JW], ps[:, :])
                prev_xbc = xbc_r

                t1 = sb.tile([128, JW], F32, tag="conv1")
                nc.gpsimd.tensor_scalar(t1[:, :], xbc_r[:, 8:8 + JW],
                                        CW("w1c"), CW("cbc"),
                                        ALU.mult, ALU.add)  # gpsimd ts ok?
                vj = sb.tile([128, JW], F32, tag="conv2")
                nc.vector.scalar_tensor_tensor(vj[:, :], xbc_r[:, 7:7 + JW],
                                               CW("w0c"), t1[:, :],
                                               ALU.mult, ALU.add)
                th_c = sbA.tile([128, JW], F32, tag="thc")
                nc.scalar.activation(th_c[:, :], vj[:, :], AF.Tanh,
                                     bias=CW("zeroc"), scale=0.5)
                xbc_j = sb.tile([128, JW], BF16, tag="xbcc")
                nc.vector.scalar_tensor_tensor(xbc_j[:, :], th_c[:, :], 1.0,
                                               vj[:, :], ALU.add, ALU.mult)

                ps_dtr = psA.tile([128, JW], F32, tag="psa")
                nc.tensor.matmul(ps_dtr[0:64, :], CWB("Edtr"),
                                 dt_all[:, c0:c0 + JW])
                dtx_j = sb2.tile([64, JW], BF16, tag="dtx")
                nc.vector.tensor_mul(dtx_j[:, :], xbc_j[0:64, :],
                                     ps_dtr[0:64, :])

                ps_y = psY.tile([64, JW], F32, tag="psy")
                for b in range(BL):
                    ps_da = psA.tile([128, JW], F32, tag="psa")
                    nc.tensor.matmul(ps_da[:, :], CW(f"Eda{b}"),
                                     da_all[:, c0:c0 + JW])
                    ps_dtx = psA.tile([128, JW], F32, tag="psa")
                    nc.tensor.matmul(ps_dtx[:, :], CWB(f"Edtx{b}"),
                                     dtx_j[:, :])
                    ps_B = psA.tile([128, JW], F32, tag="psa")
                    nc.tensor.matmul(ps_B[:, :], CWB(f"EB{b}"), xbc_j[:, :])
                    B_sb = sb2.tile([128, JW], F32, tag="Bsb")
                    nc.scalar.copy(B_sb[:, :], ps_B[:, :])
                    d1 = sb2.tile([128, JW], F32, tag="d1")
                    nc.vector.tensor_mul(d1[:, :], B_sb[:, :], ps_dtx[:, :])

                    h_bj = hpool.tile([128, JW], F32, tag="h")
                    init = 0.0 if j == 0 else hlast[b][:, 0:1]
                    nc.vector.tensor_tensor_scan(h_bj[:, :], ps_da[:, :],
                                                 d1[:, :], init,
                                                 ALU.mult, ALU.add)
                    nc.vector.tensor_copy(hlast[b][:, 0:1],
                                          h_bj[:, JW - 1:JW])

                    ps_C = psA.tile([128, JW], F32, tag="psa")
                    nc.tensor.matmul(ps_C[:, :], CWB(f"EC{b}"), xbc_j[:, :])
                    Pm = sb2.tile([128, JW], BF16, tag="Pm")
                    nc.vector.tensor_mul(Pm[:, :], h_bj[:, :], ps_C[:, :])
                    nc.tensor.matmul(ps_y[:, :], CWB(f"Wred{b}"), Pm[:, :],
                                     start=(b == 0), stop=(b == BL - 1))

                y2 = sb.tile([64, JW], F32, tag="y2")
                nc.vector.scalar_tensor_tensor(y2[:, :], xbc_j[0:64, :],
                                               CW("Dskc"), ps_y[:, :],
                                               ALU.mult, ALU.add)
                nc.vector.tensor_mul(y3_all[:, c0:c0 + JW], y2[:, :],
                                     zs_j[:, :])
                sq = sbA.tile([64, JW], BF16, tag="sq")
                nc.scalar.activation(sq[:, :], y3_all[:, c0:c0 + JW],
                                     AF.Square, bias=CW("zeroc", 64))
                ps_s = psS.tile([4, JW], F32, tag="pss")
                nc.tensor.matmul(ps_s[:, :], CWB("Wsq"), sq[:, :])
                nc.scalar.copy(ms_cp[:, c0:c0 + JW], ps_s[:, :])

            # ====== phase 2: rsqrt via exp(-0.5 ln(.))  (ACT: ln+exp) =====
            for g in range(2):
                g0 = g * (T // 2)
                gw = T // 2
                ln_t = sbA.tile([4, gw], F32, tag="ln_t")
                nc.scalar.activation(ln_t[:, :], ms_cp[:, g0:g0 + gw], AF.Ln,
                                     bias=CW("epsc", 4), scale=1.0 / DI)
                nc.scalar.activation(ms_cp[:, g0:g0 + gw], ln_t[:, :],
                                     AF.Exp, bias=CW("zeroc", 4), scale=-0.5)

            # ====== phase 3: y4, out_proj, logits (ACT: relu/copy any) ====
            for j in range(NJ):
                c0 = j * JW
                ps_r = psS.tile([64, JW], F32, tag="psr")
                nc.tensor.matmul(ps_r[:, :], CWB("E4b"),
                                 ms_cp[:, c0:c0 + JW])
                y4 = sb.tile([64, JW], BF16, tag="y4")
                nc.vector.tensor_mul(y4[:, :], y3_all[:, c0:c0 + JW],
                                     ps_r[:, :])
                ps_yo = psY.tile([64, JW], F32, tag="psyo")
                nc.tensor.matmul(ps_yo[:, :], CWB("OutP"), y4[:, :])
                hid = sbA.tile([65, JW], BF16, tag="hid")
                nc.scalar.activation(hid[0:64, :], ps_yo[:, :], AF.Relu,
                                     bias=CW("zeroc", 64))
                nc.gpsimd.memset(hid[64:65, :], 1.0)

                stage = sbA.tile([128, 256], F32, tag="stage")
                for blk in range(4):
                    ps_lg = psS.tile([128, 64], F32, tag="psr")
                    nc.tensor.matmul(ps_lg[:, :],
                                     hid[:, 128 * blk:128 * blk + 128],
                                     CWB("WoutT"))
                    if blk % 2 == 0:
                        nc.vector.tensor_copy(stage[:, 64 * blk:64 * blk + 64],
                                              ps_lg[:, :])
                    else:
                        nc.scalar.copy(stage[:, 64 * blk:64 * blk + 64],
                                       ps_lg[:, :])

                out_v = out.ap()[c0:c0 + JW, :, :].rearrange(
                    "(k t) b a -> t k (b a)", t=128)
                nc.sync.dma_start(out_v, stage[:, :].rearrange(
                    "p (k a) -> p k a", k=4))

    nc.compile()
    return nc


_CACHE = {}


def kernel(**inputs):
    obs = np.asarray(inputs["obs"], np.float32)
    blob, offs = _build_consts(
        np.asarray(inputs["W_in"], np.float32),
        np.asarray(inputs["b_in"], np.float32),
        np.asarray(inputs["in_proj"], np.float32),
        np.asarray(inputs["conv_w"], np.float32),
        np.asarray(inputs["conv_b"], np.float32),
        np.asarray(inputs["dt_bias"], np.float32),
        np.asarray(inputs["A_log"], np.float32),
        np.asarray(inputs["D_skip"], np.float32),
        np.asarray(inputs["norm_w"], np.float32),
        np.asarray(inputs["out_proj"], np.float32),
        np.asarray(inputs["W_out"], np.float32),
        np.asarray(inputs["b_out"], np.float32),
    )

    if "nc" not in _CACHE:
        _CACHE["nc"] = build_nc(offs, blob.shape[1])
    nc = _CACHE["nc"]

    # shard obs: core i gets batches [4i, 4i+4): [T, 32b, 32f] -> [(b,f)=128, T]
    obs_bft = np.ascontiguousarray(obs.transpose(1, 2, 0))  # [32b, 32f, T]
    in_maps = []
    for i in range(NCORES):
        shard = obs_bft[BL * i:BL * i + BL].reshape(128, T)
        in_maps.append({"obs_t": np.ascontiguousarray(shard).astype(ml_dtypes.bfloat16),
                        "cw": blob,
                        "cwb": blob.astype(ml_dtypes.bfloat16)})

    res = run_bass_kernel_spmd(nc, in_maps, core_ids=list(range(NCORES)))
    outs = [res.results[i]["out"] for i in range(NCORES)]
    return np.concatenate(outs, axis=1)  # [T, 32, 16]


def _install_ntff_hook():
    """Provide antenv.axon_hooks with a ctypes NTFF profile hook (missing
    in this container image) so run_bass_kernel_spmd(trace=True) works."""
    import sys as _sys
    import types as _types
    import contextlib as _ctx
    import ctypes as _ct
    if "antenv.axon_hooks" in _sys.modules:
        return
    try:
        lib = _ct.CDLL("/opt/axon/libaxon_pjrt.so")
        if not hasattr(lib, "axon_start_nrt_profile"):
            return
    except OSError:
        return
    lib.axon_start_nrt_profile.argtypes = [_ct.POINTER(_ct.c_int64),
                                           _ct.c_size_t]
    lib.axon_start_nrt_profile.restype = _ct.c_int64
    lib.axon_stop_nrt_profile.argtypes = [_ct.c_char_p]
    lib.axon_stop_nrt_profile.restype = _ct.c_int64

    @_ctx.contextmanager
    def _hook(output_dir, device_ids):
        import jax
        jax.devices()
        if device_ids:
            ids = (_ct.c_int64 * len(device_ids))(*device_ids)
            rc = lib.axon_start_nrt_profile(ids, len(device_ids))
        else:
            rc = lib.axon_start_nrt_profile(None, 0)
        if rc != 0:
            raise RuntimeError(f"axon_start_nrt_profile rc={rc}")
        try:
            yield
        finally:
            n = lib.axon_stop_nrt_profile(str(output_dir).encode())
            print(f"ntff profile: {n} file(s) -> {output_dir}")

    mod = _types.ModuleType("antenv.axon_hooks")
    mod.get_axon_ntff_profile_hook = lambda: _hook
    mod.set_axon_ntff_profile_hook = lambda h: None
    _sys.modules["antenv.axon_hooks"] = mod
    import antenv
    antenv.axon_hooks = mod


def bench(**inputs):
    """Correctness + profiled run; returns (out, exec_time_ns)."""
    out = kernel(**inputs)  # ensures _CACHE["nc"] built with same consts
    obs = np.asarray(inputs["obs"], np.float32)
    blob, offs = _build_consts(*[np.asarray(inputs[k], np.float32) for k in (
        "W_in", "b_in", "in_proj", "conv_w", "conv_b", "dt_bias", "A_log",
        "D_skip", "norm_w", "out_proj", "W_out", "b_out")])
    obs_bft = np.ascontiguousarray(obs.transpose(1, 2, 0))
    in_maps = []
    for i in range(NCORES):
        shard = obs_bft[BL * i:BL * i + BL].reshape(128, T)
        in_maps.append({"obs_t": np.ascontiguousarray(shard).astype(ml_dtypes.bfloat16),
                        "cw": blob,
                        "cwb": blob.astype(ml_dtypes.bfloat16)})
    _install_ntff_hook()
    try:
        res = run_bass_kernel_spmd(_CACHE["nc"], in_maps,
                                   core_ids=list(range(NCORES)), trace=True)
        return out, res.exec_time_ns, res
    except Exception as e:
        print("trace run failed:", repr(e)[:300])
        return out, None, None


# revision 21
# speedup vs baseline: 1.6039x; 1.2272x over previous
"""Trainium2 Bass kernel for nn_ActorAgent (Mamba2-SSD actor network).

Data-parallel over batch: 32 sequences -> 8 cores x 4 sequences.
Per-core: full sequential SSM scan via tensor_tensor_scan on
[(h,p,n)=128, T] lanes, all per-batch selection/broadcast done with
constant block lhsT matmuls on the PE.
"""

import sys
import numpy as np
import ml_dtypes

for _p in ("/opt/trn_rl_repo",):
    if _p not in sys.path:
        sys.path.append(_p)

import concourse.bass as bass
import concourse.bacc as bacc_mod
import concourse.mybir as mybir
from concourse.tile import TileContext
from concourse.bass_utils import run_bass_kernel_spmd

F32 = mybir.dt.float32
AF = mybir.ActivationFunctionType
ALU = mybir.AluOpType

T = 4096
B = 32
NCORES = 8
BL = B // NCORES          # 4 sequences per core
OBS = 32
DM = 16                   # d_model
DI = 16                   # d_inner
H = 8                     # heads
P = 2                     # headdim
N = 8                     # d_state
NA = 16                   # n_actions
NJ = 8                    # column tiles of 512
JW = 512


def _build_consts(W_in, b_in, in_proj, conv_w, conv_b, dt_bias, A_log, D_skip,
                  norm_w, out_proj, W_out, b_out):
    """Build the constant-weight blob [128, NCOLS] (fp32)."""
    A = -np.exp(A_log)

    cols = []
    offs = {}

    def add(name, m):
        # m: [K, M] -> pad partitions to 128
        k, w = m.shape
        pad = np.zeros((128, w), np.float32)
        pad[:k] = m
        offs[name] = (sum(c.shape[1] for c in cols), k, w)
        cols.append(pad)

    # xbc class-major M layout: m<64: xs (b=m//16, c=m%16); 64..96: B (b,n);
    # 96..128: C (b,n)
    def xbc_cm(m):
        if m < 64:
            return m // 16, m % 16
        if m < 96:
            return (m - 64) // 8, 16 + (m - 64) % 8
        return (m - 96) // 8, 24 + (m - 96) % 8

    # W1bd [128=(b,32f), 64=(b,16o)]
    w = np.zeros((128, 64), np.float32)
    for b in range(BL):
        w[32 * b:32 * b + 32, 16 * b:16 * b + 16] = W_in.T
    add("W1", w)

    # Wz [64=(b,16f), 64=(b,16z)]
    w = np.zeros((64, 64), np.float32)
    for b in range(BL):
        w[16 * b:16 * b + 16, 16 * b:16 * b + 16] = in_proj[:16].T
    add("Wz", w)

    # Wxbc [64=(b,16f), 128 class-major]
    w = np.zeros((64, 128), np.float32)
    for m in range(128):
        b, c = xbc_cm(m)
        w[16 * b:16 * b + 16, m] = in_proj[16 + c]
    add("Wxbc", w)

    # Wdt [64, 32=(b,8h)]
    w = np.zeros((64, 32), np.float32)
    for b in range(BL):
        w[16 * b:16 * b + 16, 8 * b:8 * b + 8] = in_proj[48:56].T
    add("Wdt", w)

    # E_dtr [32=(b,h), 64=(b,hp)] (x0.5: xs is stored as 2*silu)
    w = np.zeros((32, 64), np.float32)
    for m in range(64):
        b, hp = m // 16, m % 16
        w[8 * b + hp // 2, m] = 0.5
    add("Edtr", w)

    # per-b selectors, m=(h,p,n): h=m//16, hp=m//8, n=m%8
    for b in range(BL):
        w = np.zeros((64, 128), np.float32)   # E_dtx_b: k=(b',hp16)
        for m in range(128):
            w[16 * b + m // 8, m] = 1.0
        add(f"Edtx{b}", w)
    for b in range(BL):
        w = np.zeros((128, 128), np.float32)  # E_B_b from xbc_c rows (x0.5)
        for m in range(128):
            w[64 + 8 * b + m % 8, m] = 0.5
        add(f"EB{b}", w)
    for b in range(BL):
        w = np.zeros((128, 128), np.float32)  # E_C_b (x0.5)
        for m in range(128):
            w[96 + 8 * b + m % 8, m] = 0.5
        add(f"EC{b}", w)
    for b in range(BL):
        w = np.zeros((32, 128), np.float32)   # E_da_b from da_all [32=(b,h)]
        for m in range(128):
            w[8 * b + m // 16, m] = 1.0
        add(f"Eda{b}", w)
    for b in range(BL):
        w = np.zeros((128, 64), np.float32)   # Wred_b: sum over n -> (b,hp)
        for k in range(128):
            w[k, 16 * b + k // 8] = 1.0
        add(f"Wred{b}", w)

    # Wsq [64=(b,hp), 4]
    w = np.zeros((64, 4), np.float32)
    for k in range(64):
        w[k, k // 16] = 1.0
    add("Wsq", w)
    # E4b [4, 64]
    w = np.zeros((4, 64), np.float32)
    for m in range(64):
        w[m // 16, m] = 1.0
    add("E4b", w)
    # per-j selectors from rsq_pack [32=(j,b), .] -> [64=(b,hp), .]
    for j in range(8):
        w = np.zeros((32, 64), np.float32)
        for m in range(64):
            w[4 * j + m // 16, m] = 1.0
        add(f"E4b{j}", w)
    # OutP [64=(b,hp), 64=(b,d)] with norm_w folded
    opw = out_proj * norm_w[None, :]          # [16d, 16hp]
    w = np.zeros((64, 64), np.float32)
    for b in range(BL):
        w[16 * b:16 * b + 16, 16 * b:16 * b + 16] = opw.T
    add("OutP", w)
    # WoutT [65, 64=(b,a)]: rhs for t-major logits
    w = np.zeros((65, 64), np.float32)
    for b in range(BL):
        w[16 * b:16 * b + 16, 16 * b:16 * b + 16] = W_out.T
    w[64] = np.tile(b_out, BL)
    add("WoutT", w)

    # scalar columns
    cvec = np.array([xbc_cm(m)[1] for m in range(128)])
    add("w0c", conv_w[cvec, 0][:, None].astype(np.float32))
    add("w1c", conv_w[cvec, 1][:, None].astype(np.float32))
    add("cbc", conv_b[cvec][:, None].astype(np.float32))
    add("binc", np.tile(b_in, BL)[:, None].astype(np.float32))
    add("dtbc", np.tile(dt_bias, BL)[:, None].astype(np.float32))
    add("Ac", np.tile(A, BL)[:, None].astype(np.float32))
    add("Dskc", 0.5 * np.tile(np.repeat(D_skip, P), BL)[:, None].astype(np.float32))
    add("zeroc", np.zeros((128, 1), np.float32))
    add("epsc", np.full((128, 1), 4e-5, np.float32))
    add("onec", np.ones((128, 1), np.float32))

    blob = np.concatenate(cols, axis=1).astype(np.float32)
    return blob, offs


def build_nc(offs, ncols):
    nc = bacc_mod.Bacc()
    BF = mybir.dt.bfloat16
    obs_t = nc.declare_dram_parameter("obs_t", [128, T], BF, isOutput=False)
    cw = nc.declare_dram_parameter("cw", [128, ncols], F32, isOutput=False)
    cwb = nc.declare_dram_parameter("cwb", [128, ncols], BF, isOutput=False)
    out = nc.declare_dram_parameter("out", [T, BL, NA], F32, isOutput=True)
    BF16 = mybir.dt.bfloat16

    with TileContext(nc) as tc:
        with (
            tc.tile_pool(name="persist", bufs=1) as pp,
            tc.tile_pool(name="sb", bufs=2) as sb,
            tc.tile_pool(name="sbA", bufs=1) as sbA,
            tc.tile_pool(name="sb2", bufs=2) as sb2,
            tc.tile_pool(name="hpool", bufs=3) as hpool,
            tc.tile_pool(name="obsp", bufs=2) as obsp,
            tc.tile_pool(name="xbcp", bufs=2) as xbcp,
            tc.tile_pool(name="psA", bufs=6, space="PSUM") as psA,
            tc.tile_pool(name="psY", bufs=1, space="PSUM") as psY,
            tc.tile_pool(name="psS", bufs=1, space="PSUM") as psS,
        ):
            cwt = pp.tile([128, ncols], F32, tag="cw")
            nc.sync.dma_start(cwt[:, :], cw[:, :])
            cwbt = pp.tile([128, ncols], BF16, tag="cwb")
            nc.sync.dma_start(cwbt[:, :], cwb[:, :])

            def CW(name, nk=None):
                c0, k, w = offs[name]
                return cwt[0:(nk or k), c0:c0 + w]

            def CWB(name, nk=None):
                c0, k, w = offs[name]
                return cwbt[0:(nk or k), c0:c0 + w]

            x_all = pp.tile([64, T], BF16, tag="x_all")
            dt_all = pp.tile([32, T], BF16, tag="dt_all")
            da_all = pp.tile([32, T], BF16, tag="da_all")
            y3_all = pp.tile([64, T], BF16, tag="y3_all")
            ms_cp = pp.tile([4, T], BF16, tag="ms_cp")
            hlast = []
            for b in range(BL):
                hl_b = pp.tile([128, 1], F32, tag=f"hlast{b}")
                hlast.append(hl_b)

            # dummy matmul: folds the cwt DMA wait into PE's clock so the
            # first real matmul carries a single sync wait (walrus limit)
            ps_w = psS.tile([1, 16], F32, tag="pss")
            nc.tensor.matmul(ps_w[0:1, :], cwt[0:1, 0:1], cwt[0:1, 0:16])

            # ================= phase 0: x, dt, da  (ACT set: ln+exp) ======
            for j in range(NJ):
                c0 = j * JW
                ot = obsp.tile([128, JW], BF16, tag="obs")
                nc.sync.dma_start(ot[:, :], obs_t[:, c0:c0 + JW])
                ps = psA.tile([128, JW], F32, tag="psa")
                nc.tensor.matmul(ps[0:64, :], CWB("W1"), ot[:, :])
                nc.scalar.activation(x_all[:, c0:c0 + JW], ps[0:64, :],
                                     AF.Relu, bias=CW("binc"))
                ps = psA.tile([128, JW], F32, tag="psa")
                nc.tensor.matmul(ps[0:32, :], CWB("Wdt"),
                                 x_all[:, c0:c0 + JW])
                # softplus(v) = relu(v) + ln(1 + exp(-|v|)), v = raw + dt_bias
                rl = sbA.tile([32, JW], F32, tag="spt1")
                nc.scalar.activation(rl[:, :], ps[0:32, :],
                                     AF.Relu, bias=CW("dtbc", 32))
                ab = sbA.tile([32, JW], F32, tag="spt2")
                nc.scalar.activation(ab[:, :], ps[0:32, :],
                                     AF.Abs, bias=CW("dtbc", 32))
                et = sbA.tile([32, JW], F32, tag="spt3")
                nc.scalar.activation(et[:, :], ab[:, :], AF.Exp,
                                     bias=CW("zeroc", 32), scale=-1.0)
                lt = sbA.tile([32, JW], F32, tag="spt4")
                nc.scalar.activation(lt[:, :], et[:, :], AF.Ln,
                                     bias=CW("onec", 32))
                nc.vector.tensor_add(dt_all[:, c0:c0 + JW], rl[:, :],
                                     lt[:, :])
                adt = sbA.tile([32, JW], F32, tag="spt5")
                nc.vector.tensor_scalar_mul(adt[:, :],
                                            dt_all[:, c0:c0 + JW],
                                            CW("Ac", 32))
                nc.scalar.activation(da_all[:, c0:c0 + JW], adt[:, :],
                                     AF.Exp, bias=CW("zeroc", 32))

            # ====== phase 1: z, conv, scan, gate  (ACT set: exp+tanh) =====
            prev_xbc = None
            for j in range(NJ):
                c0 = j * JW

                ps_z = psA.tile([128, JW], F32, tag="psa")
                nc.tensor.matmul(ps_z[0:64, :], CWB("Wz"), x_all[:, c0:c0 + JW])
                th_z = sbA.tile([64, JW], F32, tag="thz")
                nc.scalar.activation(th_z[:, :], ps_z[0:64, :], AF.Tanh,
                                     bias=CW("zeroc", 64), scale=0.5)
                zs_j = sb.tile([64, JW], F32, tag="zs")
                nc.vector.scalar_tensor_tensor(zs_j[:, :], th_z[:, :], 1.0,
                                               ps_z[0:64, :], ALU.add, ALU.mult)

                xbc_r = xbcp.tile([128, 8 + JW], F32, tag="xbcraw")
                if j == 0:
                    nc.vector.memset(xbc_r[:, 0:8], 0.0)
                else:
                    nc.vector.tensor_copy(xbc_r[:, 0:8],
                                          prev_xbc[:, JW:JW + 8])
                ps = psA.tile([128, JW], F32, tag="psa")
                nc.tensor.matmul(ps[:, :], CWB("Wxbc"), x_all[:, c0:c0 + JW])
                nc.scalar.copy(xbc_r[:, 8:8 + JW], ps[:, :])
                prev_xbc = xbc_r

                t1 = sb.tile([128, JW], F32, tag="conv1")
                nc.gpsimd.tensor_scalar(t1[:, :], xbc_r[:, 8:8 + JW],
                                        CW("w1c"), CW("cbc"),
                                        ALU.mult, ALU.add)  # gpsimd ts ok?
                vj = sb.tile([128, JW], F32, tag="conv2")
                nc.vector.scalar_tensor_tensor(vj[:, :], xbc_r[:, 7:7 + JW],
                                               CW("w0c"), t1[:, :],
                                               ALU.mult, ALU.add)
                th_c = sbA.tile([128, JW], F32, tag="thc")
                nc.scalar.activation(th_c[:, :], vj[:, :], AF.Tanh,
                                     bias=CW("zeroc"), scale=0.5)
                xbc_j = sb.tile([128, JW], BF16, tag="xbcc")
                nc.vector.scalar_tensor_tensor(xbc_j[:, :], th_c[:, :], 1.0,
                                               vj[:, :], ALU.add, ALU.mult)

                ps_dtr = psA.tile([128, JW], F32, tag="psa")
                nc.tensor.matmul(ps_dtr[0:64, :], CWB("Edtr"),
                                 dt_all[:, c0:c0 + JW])
                dtx_j = sb2.tile([64, JW], BF16, tag="dtx")
                nc.vector.tensor_mul(dtx_j[:, :], xbc_j[0:64, :],
                                     ps_dtr[0:64, :])

                ps_y = psY.tile([64, JW], F32, tag="psy")
                for b in range(BL):
                    ps_da = psA.tile([128, JW], F32, tag="psa")
                    nc.tensor.matmul(ps_da[:, :], CWB(f"Eda{b}"),
                                     da_all[:, c0:c0 + JW])
                    ps_dtx = psA.tile([128, JW], F32, tag="psa")
                    nc.tensor.matmul(ps_dtx[:, :], CWB(f"Edtx{b}"),
                                     dtx_j[:, :])
                    ps_B = psA.tile([128, JW], F32, tag="psa")
                    nc.tensor.matmul(ps_B[:, :], CWB(f"EB{b}"), xbc_j[:, :])
                    B_sb = sb2.tile([128, JW], F32, tag="Bsb")
                    nc.scalar.copy(B_sb[:, :], ps_B[:, :])
                    d1 = sb2.tile([128, JW], F32, tag="d1")
                    nc.vector.tensor_mul(d1[:, :], B_sb[:, :], ps_dtx[:, :])

                    h_bj = hpool.tile([128, JW], F32, tag="h")
                    init = 0.0 if j == 0 else hlast[b][:, 0:1]
                    nc.vector.tensor_tensor_scan(h_bj[:, :], ps_da[:, :],
                                                 d1[:, :], init,
                                                 ALU.mult, ALU.add)
                    nc.vector.tensor_copy(hlast[b][:, 0:1],
                                          h_bj[:, JW - 1:JW])

                    ps_C = psA.tile([128, JW], F32, tag="psa")
                    nc.tensor.matmul(ps_C[:, :], CWB(f"EC{b}"), xbc_j[:, :])
                    Pm = sb2.tile([128, JW], BF16, tag="Pm")
                    nc.vector.tensor_mul(Pm[:, :], h_bj[:, :], ps_C[:, :])
                    nc.tensor.matmul(ps_y[:, :], CWB(f"Wred{b}"), Pm[:, :],
                                     start=(b == 0), stop=(b == BL - 1))

                y2 = sb.tile([64, JW], F32, tag="y2")
                nc.vector.scalar_tensor_tensor(y2[:, :], xbc_j[0:64, :],
                                               CW("Dskc"), ps_y[:, :],
                                               ALU.mult, ALU.add)
                nc.vector.tensor_mul(y3_all[:, c0:c0 + JW], y2[:, :],
                                     zs_j[:, :])
                sq = sbA.tile([64, JW], BF16, tag="sq")
                nc.scalar.activation(sq[:, :], y3_all[:, c0:c0 + JW],
                                     AF.Square, bias=CW("zeroc", 64))
                ps_s = psS.tile([4, JW], F32, tag="pss")
                nc.tensor.matmul(ps_s[:, :], CWB("Wsq"), sq[:, :])
                nc.scalar.copy(ms_cp[:, c0:c0 + JW], ps_s[:, :])

            # ====== phase 2: rsqrt via exp(-0.5 ln(.))  (ACT: ln+exp) =====
            for g in range(2):
                g0 = g * (T // 2)
                gw = T // 2
                ln_t = sbA.tile([4, gw], F32, tag="ln_t")
                nc.scalar.activation(ln_t[:, :], ms_cp[:, g0:g0 + gw], AF.Ln,
                                     bias=CW("epsc", 4), scale=1.0 / DI)
                nc.scalar.activation(ms_cp[:, g0:g0 + gw], ln_t[:, :],
                                     AF.Exp, bias=CW("zeroc", 4), scale=-0.5)

            # ====== phase 3: y4, out_proj, logits (ACT: relu/copy any) ====
            for j in range(NJ):
                c0 = j * JW
                ps_r = psA.tile([128, JW], F32, tag="psa")
                nc.tensor.matmul(ps_r[0:64, :], CWB("E4b"),
                                 ms_cp[:, c0:c0 + JW])
                y4 = sb.tile([64, JW], BF16, tag="y4")
                nc.vector.tensor_mul(y4[:, :], y3_all[:, c0:c0 + JW],
                                     ps_r[0:64, :])
                ps_yo = psA.tile([128, JW], F32, tag="psa")
                nc.tensor.matmul(ps_yo[0:64, :], CWB("OutP"), y4[:, :])
                hid = sbA.tile([65, JW], BF16, tag="hid")
                nc.scalar.activation(hid[0:64, :], ps_yo[0:64, :], AF.Relu,
                                     bias=CW("zeroc", 64))
                nc.gpsimd.memset(hid[64:65, :], 1.0)

                stage = sbA.tile([128, 256], F32, tag="stage")
                for blk in range(4):
                    ps_lg = psA.tile([128, JW], F32, tag="psa")
                    nc.tensor.matmul(ps_lg[:, 0:64],
                                     hid[:, 128 * blk:128 * blk + 128],
                                     CWB("WoutT"))
                    if blk % 2 == 0:
                        nc.vector.tensor_copy(stage[:, 64 * blk:64 * blk + 64],
                                              ps_lg[:, 0:64])
                    else:
                        nc.scalar.copy(stage[:, 64 * blk:64 * blk + 64],
                                       ps_lg[:, 0:64])

                out_v = out.ap()[c0:c0 + JW, :, :].rearrange(
                    "(k t) b a -> t k (b a)", t=128)
                nc.sync.dma_start(out_v, stage[:, :].rearrange(
                    "p (k a) -> p k a", k=4))

    nc.compile()
    return nc


_CACHE = {}


def kernel(**inputs):
    obs = np.asarray(inputs["obs"], np.float32)
    blob, offs = _build_consts(
        np.asarray(inputs["W_in"], np.float32),
        np.asarray(inputs["b_in"], np.float32),
        np.asarray(inputs["in_proj"], np.float32),
        np.asarray(inputs["conv_w"], np.float32),
        np.asarray(inputs["conv_b"], np.float32),
        np.asarray(inputs["dt_bias"], np.float32),
        np.asarray(inputs["A_log"], np.float32),
        np.asarray(inputs["D_skip"], np.float32),
        np.asarray(inputs["norm_w"], np.float32),
        np.asarray(inputs["out_proj"], np.float32),
        np.asarray(inputs["W_out"], np.float32),
        np.asarray(inputs["b_out"], np.float32),
    )

    if "nc" not in _CACHE:
        _CACHE["nc"] = build_nc(offs, blob.shape[1])
    nc = _CACHE["nc"]

    # shard obs: core i gets batches [4i, 4i+4): [T, 32b, 32f] -> [(b,f)=128, T]
    obs_bft = np.ascontiguousarray(obs.transpose(1, 2, 0))  # [32b, 32f, T]
    in_maps = []
    for i in range(NCORES):
        shard = obs_bft[BL * i:BL * i + BL].reshape(128, T)
        in_maps.append({"obs_t": np.ascontiguousarray(shard).astype(ml_dtypes.bfloat16),
                        "cw": blob,
                        "cwb": blob.astype(ml_dtypes.bfloat16)})

    res = run_bass_kernel_spmd(nc, in_maps, core_ids=list(range(NCORES)))
    outs = [res.results[i]["out"] for i in range(NCORES)]
    return np.concatenate(outs, axis=1)  # [T, 32, 16]


def _install_ntff_hook():
    """Provide antenv.axon_hooks with a ctypes NTFF profile hook (missing
    in this container image) so run_bass_kernel_spmd(trace=True) works."""
    import sys as _sys
    import types as _types
    import contextlib as _ctx
    import ctypes as _ct
    if "antenv.axon_hooks" in _sys.modules:
        return
    try:
        lib = _ct.CDLL("/opt/axon/libaxon_pjrt.so")
        if not hasattr(lib, "axon_start_nrt_profile"):
            return
    except OSError:
        return
    lib.axon_start_nrt_profile.argtypes = [_ct.POINTER(_ct.c_int64),
                                           _ct.c_size_t]
    lib.axon_start_nrt_profile.restype = _ct.c_int64
    lib.axon_stop_nrt_profile.argtypes = [_ct.c_char_p]
    lib.axon_stop_nrt_profile.restype = _ct.c_int64

    @_ctx.contextmanager
    def _hook(output_dir, device_ids):
        import jax
        jax.devices()
        if device_ids:
            ids = (_ct.c_int64 * len(device_ids))(*device_ids)
            rc = lib.axon_start_nrt_profile(ids, len(device_ids))
        else:
            rc = lib.axon_start_nrt_profile(None, 0)
        if rc != 0:
            raise RuntimeError(f"axon_start_nrt_profile rc={rc}")
        try:
            yield
        finally:
            n = lib.axon_stop_nrt_profile(str(output_dir).encode())
            print(f"ntff profile: {n} file(s) -> {output_dir}")

    mod = _types.ModuleType("antenv.axon_hooks")
    mod.get_axon_ntff_profile_hook = lambda: _hook
    mod.set_axon_ntff_profile_hook = lambda h: None
    _sys.modules["antenv.axon_hooks"] = mod
    import antenv
    antenv.axon_hooks = mod


def bench(**inputs):
    """Correctness + profiled run; returns (out, exec_time_ns)."""
    out = kernel(**inputs)  # ensures _CACHE["nc"] built with same consts
    obs = np.asarray(inputs["obs"], np.float32)
    blob, offs = _build_consts(*[np.asarray(inputs[k], np.float32) for k in (
        "W_in", "b_in", "in_proj", "conv_w", "conv_b", "dt_bias", "A_log",
        "D_skip", "norm_w", "out_proj", "W_out", "b_out")])
    obs_bft = np.ascontiguousarray(obs.transpose(1, 2, 0))
    in_maps = []
    for i in range(NCORES):
        shard = obs_bft[BL * i:BL * i + BL].reshape(128, T)
        in_maps.append({"obs_t": np.ascontiguousarray(shard).astype(ml_dtypes.bfloat16),
                        "cw": blob,
                        "cwb": blob.astype(ml_dtypes.bfloat16)})
    _install_ntff_hook()
    try:
        res = run_bass_kernel_spmd(_CACHE["nc"], in_maps,
                                   core_ids=list(range(NCORES)), trace=True)
        return out, res.exec_time_ns, res
    except Exception as e:
        print("trace run failed:", repr(e)[:300])
        return out, None, None
